# revision 26
# baseline (speedup 1.0000x reference)
"""Trainium2 Bass/Tile kernel for BasicCondConvBlock (E=1):
two CondConv1d(k=3,pad=1)+BN(eval)+LeakyReLU(0.1) blocks + MaxPool1d(2).

With a single expert, CondConv reduces to y_i = r_i * (conv(x_i, W) + b)
with a shared weight: conv runs on the TensorEngine as shifted matmuls
accumulated in PSUM, and routing r_i + conv bias + BatchNorm fold into one
per-(sample,channel) affine applied at PSUM drain:
    out = LeakyReLU( (r_i*s_c) * z + (r_i*b_c*s_c + be_c - rm_c*s_c) )

Key structure decisions (all measured on hardware):
- Block-1 routing depends only on x, so r_1 and the fused scale/bias
  columns are computed on the host and shipped with the weights.
- Conv data (x, w1, w2, y1) is bf16: same 1 cycle/row PE rate as fp32r,
  but half the DMA bytes, half the LDWEIGHTS time, and the affine +
  routing path stays fp32 so the end-to-end error stays ~1e-3.
- Block-1 packs taps 0+1 into the K dim (host ships x duplicated with a
  1-column shift on partitions 64:127), so conv-1 is 2 matmuls per
  512-col tile (K=128 + K=64) instead of 3 half-empty K=64 ones.  The
  PE clock only steps up (~1.2 -> 2.4 GHz window) after sustained
  full-width matmul activity, so K=128 from the very start matters.
- One continuous matmul stream: warmup matmuls on a zeroed tile cover
  the x-DMA latency; routing-2's tiny matmuls are placed where their
  inputs are long since ready.
- Bulk DMAs span all 128 partitions (64-partition DMAs run at half
  rate).  First x chunks split across the scalar ring (~154 GB/s), sync
  ring (~100 GB/s) and SWDGE (~125 GB/s) so conv starts after ~0.25 MiB
  and later samples stream in behind.

Epilogue: block-1 drains are ScalarE Prelu(affine)+row-sum (1-in-4 on
VectorE); block-2 alternates per PSUM tile between (a) VectorE max-pool
from PSUM then half-width ScalarE Prelu(affine) (exact: scale>0 keeps
affine+Prelu monotone) and (b) full-width ScalarE Prelu(affine) then
VectorE SBUF pool.  Routing-2 uses the fused drain row-sums, a tiny
Copy+accum, a 1-column fp32 matmul and a sigmoid.

Sharding: pure data parallel over batch (32 samples -> 4 per core x 8).
"""

import numpy as np

N_CORES = 8
B, CIN, W = 32, 64, 2048
C1, C2 = 128, 256
BL = B // N_CORES  # samples per core
EPS = 1e-5
SLOPE = 0.1
WT = 512           # conv output tile width (one PSUM bank of fp32)
W2T = 2 * WT       # PSUM tiles span two banks; drains amortize per-op overhead
WO = W // 2        # pooled output width
WX = W + 4         # xdup row width (pad + shift headroom)
HX = 1026          # first-half split: covers conv-1 output cols 0:1024

# wb1 (bf16): [0:128] = [tap0; tap1] stacked on K, [128:256] = tap2 in
# rows 64:128.  wb2 (bf16): w2 taps, tap-major.  wpkc (f32): 7 per-channel
# consts (s2a, s2b, t12a, t12b, t22a, t22b, fcb2), sc1[BL], bi1[BL],
# fcw2/W replicated x128.
NB1 = 2 * C1
W2C = 3 * C2
CPK = 0
SC1 = 8
BI1 = SC1 + BL
FC2 = 16
NWC = FC2 + C1

NWARM = 5          # PE warmup matmuls (p-state ramp) before x arrives

TRACE = False
LAST_RESULT = None

_built = None


def _build():
    global _built
    if _built is not None:
        return _built

    import concourse.bacc as bacc
    import concourse.mybir as mybir
    from concourse import tile
    from contextlib import ExitStack

    f32 = mybir.dt.float32
    bf16 = mybir.dt.bfloat16
    Alu = mybir.AluOpType
    Act = mybir.ActivationFunctionType
    Ax = mybir.AxisListType

    nc = bacc.Bacc("TRN2", target_bir_lowering=False, debug=False)

    xd = nc.declare_dram_parameter("x", [BL, C1, WX], bf16, isOutput=False)
    wb1d = nc.declare_dram_parameter("wb1", [C1, NB1], bf16, isOutput=False)
    wb2d = nc.declare_dram_parameter("wb2", [C1, W2C], bf16, isOutput=False)
    wcd = nc.declare_dram_parameter("wpkc", [C1, NWC], f32, isOutput=False)
    od = nc.declare_dram_parameter("out", [BL, C2, WO], f32, isOutput=True)
    x_ap, wb1_ap, wb2_ap, wc_ap, o_ap = (
        xd.ap(), wb1d.ap(), wb2d.ap(), wcd.ap(), od.ap())

    with tile.TileContext(nc) as tc:
        with ExitStack() as ctx:
            consts = ctx.enter_context(tc.tile_pool(name="consts", bufs=1))
            xpool = ctx.enter_context(tc.tile_pool(name="xp", bufs=BL))
            y1pool = ctx.enter_context(tc.tile_pool(name="y1p", bufs=BL))
            pmp = ctx.enter_context(tc.tile_pool(name="pmp", bufs=6))
            outp = ctx.enter_context(tc.tile_pool(name="outp", bufs=3))
            small = ctx.enter_context(tc.tile_pool(name="small", bufs=1))
            psum = ctx.enter_context(tc.tile_pool(name="psum", bufs=3, space="PSUM"))
            aux = ctx.enter_context(tc.tile_pool(name="aux", bufs=2, space="PSUM"))

            # --- input DMAs (queue rates: scalar ring ~154 GB/s, sync ring
            # ~100 GB/s, SWDGE ~125 GB/s; bulk transfers span 128 partitions).
            # Queue order matters: x0h0 leads the scalar ring so conv starts
            # as early as possible.
            xs = [xpool.tile([C1, WX], bf16, tag="xt", name=f"xt{s}")
                  for s in range(BL)]

            def xhalf(s, h):
                sl = slice(0, HX) if h == 0 else slice(HX, WX)
                return (xs[s][:, sl], x_ap[s : s + 1]
                        .rearrange("s c w -> (s c) w")[:, sl])

            # Spread x across all three queues so no sample waits behind
            # another: sync ring (starts early, no act-table) takes x0h0 +
            # sample 2; SWDGE takes sample 1 + w2; the scalar ring (pays the
            # act-table load first) takes the small weights, x0h1, sample 3.
            for s, h in ((0, 0), (2, 0), (2, 1)):
                o_, i_ = xhalf(s, h)
                nc.sync.dma_start(out=o_, in_=i_)
            wb1s = consts.tile([C1, NB1], bf16)
            nc.scalar.dma_start(out=wb1s[:], in_=wb1_ap[:])
            o_, i_ = xhalf(0, 1)
            nc.scalar.dma_start(out=o_, in_=i_)
            wcs = consts.tile([C1, NWC], f32)
            nc.scalar.dma_start(out=wcs[:], in_=wc_ap[:])
            for s, h in ((3, 0), (3, 1)):
                o_, i_ = xhalf(s, h)
                nc.scalar.dma_start(out=o_, in_=i_)
            for s, h in ((1, 0), (1, 1)):
                o_, i_ = xhalf(s, h)
                nc.gpsimd.dma_start(out=o_, in_=i_)
            wb2s = consts.tile([C1, W2C], bf16)
            nc.gpsimd.dma_start(out=wb2s[:], in_=wb2_ap[:])

            # zero/warmup tile: pads source + warmup matmul operands
            warm = consts.tile([C1, 8 + WT], bf16)
            nc.vector.memset(warm[:], 0.0)

            y1s = [y1pool.tile([C1, W + 2], bf16, tag="y1", name=f"y1{s}")
                   for s in range(BL)]
            for s in range(BL):
                # zero the two padding columns
                nc.vector.scalar_tensor_tensor(
                    y1s[s][:, 0 : W + 2 : W + 1],
                    warm[:, 0:2], 0.0, warm[:, 0:2], Alu.mult, Alu.mult,
                )

            # PE warmup: ramp the p-state while x is still in flight
            for i in range(NWARM):
                wp = aux.tile([C1, WT], f32, tag="aux", name=f"warm{i}")
                nc.tensor.matmul(
                    wp[:], warm[:, 0:C1], warm[:, 8 : 8 + WT],
                    start=True, stop=True,
                )

            s1acc = small.tile([C1, 2 * BL], f32)
            ssum = small.tile([C1, BL], f32)
            racc = small.tile([C1, 2], f32)
            rbc2 = small.tile([C1, BL], f32)
            sc2 = small.tile([C1, 2 * BL], f32)
            bi2 = small.tile([C1, 2 * BL], f32)

            def cpk(j):
                return wcs[:, CPK + j : CPK + j + 1]

            def sc1col(s):
                return wcs[:, SC1 + s : SC1 + s + 1]

            def bi1col(s):
                return wcs[:, BI1 + s : BI1 + s + 1]

            # ---- block 1: tap-packed conv, one continuous matmul stream.
            # Per 512-col tile: K=128 matmul covers taps 0+1 (x and its
            # 1-col shift stacked on partitions), K=64 matmul adds tap 2.
            for s in range(BL):
                for d in range(2):
                    zp = psum.tile([C1, W2T], f32, tag="zp")
                    for u in range(2):
                        c0 = d * W2T + u * WT
                        nc.tensor.matmul(
                            zp[:, u * WT : (u + 1) * WT],
                            wb1s[:, 0:C1],
                            xs[s][:, c0 : c0 + WT],
                            start=True, stop=False,
                        )
                        # tap 2: rows 0:64 of the weight column block are
                        # zero, so run it K=128 anyway — the PE clock only
                        # ramps up under sustained full-width matmuls
                        nc.tensor.matmul(
                            zp[:, u * WT : (u + 1) * WT],
                            wb1s[:, C1 : C1 + C1],
                            xs[s][:, c0 + 1 : c0 + 1 + WT],
                            start=False, stop=True,
                        )
                    acc = s1acc[:, 2 * s + d : 2 * s + d + 1]
                    dst = y1s[s][:, 1 + W2T * d : 1 + W2T * (d + 1)]
                    if s % 2 == 1 and d == 1:
                        # VectorE drain for 1-in-4 tiles: balances engine
                        # load; bf16 intermediate doubles DVE throughput
                        ytmp = pmp.tile([C1, W2T], bf16, tag="ytmp")
                        nc.vector.tensor_scalar(
                            ytmp[:], zp[:], sc1col(s), bi1col(s),
                            Alu.mult, Alu.add,
                        )
                        nc.vector.scalar_tensor_tensor(
                            dst, ytmp[:], SLOPE, ytmp[:], Alu.mult, Alu.max,
                            accum_out=acc,
                        )
                    else:
                        # ScalarE drain: fused Prelu(affine) + row-sum
                        nc.scalar.activation(
                            dst, zp[:], Act.Prelu,
                            bias=bi1col(s), scale=sc1col(s),
                            alpha=SLOPE, accum_out=acc,
                        )
                # fold the two per-tile row-sums into this sample's ssum col
                nc.scalar.activation(
                    racc[:, 0:2], s1acc[:, 2 * s : 2 * (s + 1)], Act.Copy,
                    accum_out=ssum[:, s : s + 1],
                )

            # routing-2 chain for sample s: 1-col fp32 matmul against the
            # replicated fc weight -> sigmoid -> fused scale/bias columns.
            # Emitted late so the TensorE never waits on drain row-sums.
            def r2block(s):
                lg = aux.tile([C1, WT], f32, tag="aux", name=f"lg{s}")
                nc.tensor.matmul(
                    lg[:, 0:1],
                    wcs[:, FC2 : FC2 + C1],
                    ssum[:, s : s + 1],
                    start=True, stop=True,
                )
                nc.scalar.activation(
                    rbc2[:, s : s + 1], lg[:, 0:1], Act.Sigmoid,
                    bias=cpk(6), scale=1.0,
                )
                for c in range(2):
                    nc.vector.tensor_scalar(
                        sc2[:, c * BL + s : c * BL + s + 1], cpk(c),
                        rbc2[:, s : s + 1], None, Alu.mult,
                    )
                    nc.vector.scalar_tensor_tensor(
                        bi2[:, c * BL + s : c * BL + s + 1], cpk(2 + c),
                        rbc2[:, s : s + 1], cpk(4 + c), Alu.mult, Alu.add,
                    )

            # ---- block 2: conv(128->256); alternating PSUM-drain structure
            def b2unit(s, c, split_dma=False, dma_eng=None, a_only=False):
                dma_eng = dma_eng or nc.gpsimd
                ot = outp.tile([C1, WO], f32, tag="ot")
                sc_col = sc2[:, c * BL + s : c * BL + s + 1]
                bi_col = bi2[:, c * BL + s : c * BL + s + 1]
                for d in range(2):
                    zp2 = psum.tile([C1, W2T], f32, tag="zp")
                    for u in range(2):
                        c0 = d * W2T + u * WT
                        for k in range(3):
                            nc.tensor.matmul(
                                zp2[:, u * WT : (u + 1) * WT],
                                wb2s[:, k * C2 + C1 * c : k * C2 + C1 * c + C1],
                                y1s[s][:, c0 + k : c0 + k + WT],
                                start=(k == 0), stop=(k == 2),
                            )
                    if d == 0 or a_only:
                        # VectorE drains PSUM: one-input 3D-AP max-pool, then
                        # ScalarE Prelu(affine) at half width
                        pm = pmp.tile([C1, WT], f32, tag="pm")
                        nc.vector.tensor_reduce(
                            pm[:], zp2[:].rearrange("p (a b) -> p a b", b=2),
                            axis=Ax.X, op=Alu.max,
                        )
                        nc.scalar.activation(
                            ot[:, d * WT : (d + 1) * WT], pm[:], Act.Prelu,
                            bias=bi_col, scale=sc_col, alpha=SLOPE,
                        )
                    else:
                        # ScalarE drains PSUM: full-width Prelu(affine), then
                        # VectorE pools from SBUF (bf16: 2x DVE throughput)
                        yw = pmp.tile([C1, W2T], bf16, tag="yw")
                        nc.scalar.activation(
                            yw[:], zp2[:], Act.Prelu,
                            bias=bi_col, scale=sc_col, alpha=SLOPE,
                        )
                        nc.vector.tensor_tensor(
                            ot[:, WT:WO], yw[:, 0:W2T:2], yw[:, 1:W2T:2],
                            Alu.max,
                        )
                    if split_dma:
                        # last unit: ship each pooled half as soon as it
                        # lands to shorten the end-of-kernel DMA tail
                        dma_eng.dma_start(
                            out=o_ap[s, C1 * c : C1 * (c + 1),
                                     d * WT : (d + 1) * WT],
                            in_=ot[:, d * WT : (d + 1) * WT],
                        )
                if not split_dma:
                    dma_eng.dma_start(
                        out=o_ap[s, C1 * c : C1 * (c + 1), :], in_=ot[:]
                    )

            r2block(0)
            b2unit(0, 0)
            r2block(1)
            b2unit(0, 1)
            r2block(2)
            b2unit(1, 0)
            r2block(3)
            b2unit(1, 1)
            b2unit(2, 0)
            b2unit(2, 1)
            # final outputs ride the (long idle) rings so they neither queue
            # behind earlier SWDGE outs nor pay its slower transfer rate
            b2unit(3, 0, split_dma=True, dma_eng=nc.sync)
            b2unit(3, 1, split_dma=True, dma_eng=nc.scalar, a_only=True)

    nc.compile()
    _built = nc
    return nc


def _pack_inputs(x, w1, b1, fcw1, fcb1, g1, be1, rm1, rv1,
                 w2, b2, fcw2, fcb2, g2, be2, rm2, rv2):
    import ml_dtypes
    f = np.float32
    bf = ml_dtypes.bfloat16
    s1 = (g1 / np.sqrt(rv1 + EPS)).astype(f)
    s2 = (g2 / np.sqrt(rv2 + EPS)).astype(f)
    assert np.all(s2 > 0)  # max-pool before affine+Prelu needs monotonicity
    t11, t21 = (b1[0] * s1).astype(f), (be1 - rm1 * s1).astype(f)
    t12, t22 = (b2[0] * s2).astype(f), (be2 - rm2 * s2).astype(f)

    # block-1 routing is a pure function of the inputs: fold it on the host
    r1 = 1.0 / (1.0 + np.exp(-(x.mean(axis=-1) @ fcw1[0] + fcb1[0])))  # (B,)
    sc1 = r1[:, None] * s1[None, :]                    # (B, C1)
    bi1 = r1[:, None] * t11[None, :] + t21[None, :]    # (B, C1)

    w1t = w1[0].transpose(1, 2, 0).reshape(CIN, 3 * C1).astype(f)
    w2t = w2[0].transpose(1, 2, 0).reshape(C1, 3 * C2).astype(f)

    # wb1: [tap0; tap1] stacked on K in cols 0:128, tap2 in rows 64:128 of
    # cols 128:256
    wb1 = np.zeros((C1, NB1), bf)
    wb1[0:CIN, 0:C1] = w1t[:, 0:C1]
    wb1[CIN:C1, 0:C1] = w1t[:, C1 : 2 * C1]
    wb1[CIN:C1, C1:NB1] = w1t[:, 2 * C1 : 3 * C1]
    wb2 = w2t.astype(bf)

    wpkc = np.zeros((C1, NWC), f)
    for j, col in enumerate([s2[:C1], s2[C1:], t12[:C1], t12[C1:],
                             t22[:C1], t22[C1:], np.full(C1, fcb2[0], f)]):
        wpkc[:, CPK + j] = col
    wpkc[:, FC2:FC2 + C1] = (fcw2[0] / W)[:, None]

    # x duplicated with a 1-col shift on partitions 64:127 (tap packing):
    # rows 0:64 = x padded (data at cols 1:2049), rows 64:128 = x at cols
    # 0:2048 (i.e. the same data shifted left by one).
    xdup = np.zeros((B, C1, WX), bf)
    xdup[:, 0:CIN, 1 : W + 1] = x
    xdup[:, CIN:C1, 0:W] = x

    maps = []
    for i in range(N_CORES):
        sl = slice(i * BL, (i + 1) * BL)
        wc = wpkc.copy()
        wc[:, SC1:SC1 + BL] = sc1[sl].T
        wc[:, BI1:BI1 + BL] = bi1[sl].T
        maps.append({
            "wb1": wb1, "wb2": wb2, "wpkc": wc,
            "x": np.ascontiguousarray(xdup[sl]),
        })
    return maps


def _enable_trace():
    """Register the NTFF profile hook (absent antenv.axon_hooks on this image)
    and stub out the S3 artifact upload so trace=True works locally."""
    import sys
    import types

    import concourse.bass_utils as bu

    bu.upload_artifacts = lambda tmpdir: tmpdir
    if "antenv.axon_hooks" not in sys.modules:
        import antenv
        from trn_agent_boot.trn_boot import _ntff_profile_via_ctypes

        hooks = types.ModuleType("antenv.axon_hooks")
        _store = {"hook": _ntff_profile_via_ctypes("/opt/axon/libaxon_pjrt.so")}
        hooks.set_axon_ntff_profile_hook = lambda h: _store.__setitem__("hook", h)
        hooks.get_axon_ntff_profile_hook = lambda: _store["hook"]
        sys.modules["antenv.axon_hooks"] = hooks
        antenv.axon_hooks = hooks


def kernel(**inputs):
    global LAST_RESULT
    from concourse.bass_utils import run_bass_kernel_spmd

    if TRACE:
        _enable_trace()
    nc = _build()
    in_maps = _pack_inputs(**inputs)
    res = run_bass_kernel_spmd(nc, in_maps, list(range(N_CORES)), trace=TRACE)
    LAST_RESULT = res
    return np.concatenate([r["out"] for r in res.results], axis=0)
